# revision 45
# baseline (speedup 1.0000x reference)
"""Mixed-score multi-head attention Trainium2 kernel (v2).

Sharding: 8 cores = 4 batches x 2 head-quads. Each core computes, for its
batch b and its 4 heads, the full attention and a PARTIAL output projection
(its heads' slice of the recombine matmul). Host sums the two partials per
batch.

Per-core layout (H4 = 4 local heads, q = 512, k = 512):
- hidden pre-relu tiles [(s4, k32) = 128 partitions, q = 512] per (head, B, sc)
  built by ONE folded K=64 matmul into PSUM per tile:
    lhsT = KBPA band [K-block bcast over (sc,s4) ; b_s/a_s delta pattern]
    rhs  = X band    [Q_head rows ; cost.T rows]      (cols indexed by Bb)
  Folding the cost affine into the dot matmul removes one matmul + one
  semaphore hop from every PSUM-bank round-trip chain, which is what sets
  the round period (the HAM clock gate keeps the PE at 1.2 GHz for most of
  this kernel, so chain latency dominates).
- relu evac PSUM->SBUF, one op per tile; j=0,1 on ACT, j=2,3 on DVE:
    ACT tiles: relu(a*x + c)            (scale/bias per-partition APs)
    DVE tiles: max(sign(a)*x, -c/|a|)   (tensor_scalar mult/max, per-part APs)
  mix2 weights per tile form: ACT: w ; DVE: w*|a| (constant folds out of
  softmax since it is uniform over k within a head). The steady loop is
  evac-bound: ACT ~= 2 relu + exp/4, DVE ~= 2 tensor_scalar per round.
- mix2: col-packed [K=128, M=32] matmuls -> scores^T [(4h,32k), q] PSUM
- exp (no max subtraction; |scores| < 3) -> E in SBUF
- AV: per head [K=32, M=32] matmuls with replicated V, accumulated over B
- sumexp via [K=128, M=4] head-sum pattern matmul, accumulated over B
- Zrecip (approx) -> broadcast matmul -> normalize -> bf16 output proj.

Startup: operand bands stream over all three DMA rings (SP/GpSimd/ACT) in
Bb-quarter chunks; lane-aligned band pieces (j0/j1 after an (h0,h2,h1,h3)
projection-column permute, X23 cost rows) are built by DVE broadcast
copies, interleaved into the early round stream to fill DVE idle gaps.
"""

import os
import sys
import numpy as np
import ml_dtypes

import concourse.bacc as bacc
import concourse.mybir as mybir
import concourse.tile as tile
from concourse.bass_utils import run_bass_kernel_spmd


def _install_ntff_hook():
    """Provide antenv.axon_hooks (absent in this image) so trace=True can
    capture NTFF profiles via the injected libaxon_pjrt.so C ABI."""
    if "antenv.axon_hooks" in sys.modules:
        return
    import types
    import ctypes
    import contextlib

    so_path = "/opt/axon/libaxon_pjrt.so"
    hook = None
    if os.path.exists(so_path):
        lib = ctypes.CDLL(so_path)
        if hasattr(lib, "axon_start_nrt_profile"):
            lib.axon_start_nrt_profile.argtypes = [
                ctypes.POINTER(ctypes.c_int64), ctypes.c_size_t]
            lib.axon_start_nrt_profile.restype = ctypes.c_int64
            lib.axon_stop_nrt_profile.argtypes = [ctypes.c_char_p]
            lib.axon_stop_nrt_profile.restype = ctypes.c_int64

            @contextlib.contextmanager
            def _hook(output_dir, device_ids):
                import jax
                jax.devices()
                if device_ids:
                    ids = (ctypes.c_int64 * len(device_ids))(*device_ids)
                    rc = lib.axon_start_nrt_profile(ids, len(device_ids))
                else:
                    rc = lib.axon_start_nrt_profile(None, 0)
                if rc != 0:
                    raise RuntimeError(f"axon_start_nrt_profile rc={rc}")
                try:
                    yield
                finally:
                    n = lib.axon_stop_nrt_profile(str(output_dir).encode())
                    print(f"profile: {n} file(s) written to {output_dir}",
                          file=sys.stderr)
            hook = _hook
    mod = types.ModuleType("antenv.axon_hooks")
    mod.get_axon_ntff_profile_hook = lambda: hook
    mod.set_axon_ntff_profile_hook = lambda h: None
    sys.modules["antenv.axon_hooks"] = mod

f32 = mybir.dt.float32
bf16 = mybir.dt.bfloat16
MM_FAST = os.environ.get("MSK_MM_DT", "bf16") == "bf16"
fmm = bf16 if MM_FAST else f32
AF = mybir.ActivationFunctionType
ALU = mybir.AluOpType

B_, L, D, H, DK, MS = 4, 512, 256, 8, 32, 16
NB = 16          # number of 32-wide k blocks
NSC = 4          # number of s-chunks (4 s values each)

_compiled = {}
_last_results = None


def _act_form(j, sc):
    """True if tile (j, sc) is evacuated by the scalar engine (ACT form)."""
    return j < 2


NFILL = int(os.environ.get("MSK_NFILL", "0"))
MMFILL = int(os.environ.get("MSK_MMFILL", "0"))


# --------------------------------------------------------------------------
# device program
# --------------------------------------------------------------------------
def build_program():
    nc = bacc.Bacc("TRN2", target_bir_lowering=False, debug=False)

    def din(name, shape, dt=f32):
        return nc.dram_tensor(name, list(shape), dt, kind="ExternalInput").ap()

    qT = din("qT", (2, 128, 512), fmm)       # queries[b].T, D-chunked (bf16)
    costb = din("costb", (32, NB * 512), fmm)     # cost.T 32-row blocks: [kk, (Bb, q)]
    wk = din("wk", (2, 128, 256), fmm)       # Wk D-chunked (quad cols + zero pad)
    wq = din("wq", (2, 128, 256), fmm)       # Wq/sqrt(DK)
    wv = din("wv", (2, 128, 128), fmm)       # Wv cols of this quad
    wo = din("wo", (128, 256), fmm)          # Wo rows of this quad
    bpatx = din("bpatx", (4, 32, NB * NSC * 128), fmm)  # affine lhsT rows, cols (Bb, sc, 128)
    wpat = din("wpat", (NSC, 128, 128), fmm)  # mix2 lhsT per sc: cols 32j.. for head j
    evec = din("evec", (128, 36))            # evac vecs: cols 2*(sc*4+j) = scale/sgn,
    #                                          +1 = bias/thresh
    spat = din("spat", (128, 4), fmm)        # sumexp head-sum pattern
    zpat = din("zpat", (128, 128))           # Zrecip broadcast pattern (rows 0-3)
    out_d = nc.dram_tensor("out", [512, 256], f32, kind="ExternalOutput").ap()

    with tile.TileContext(nc) as tc:
        _build(nc, tc, qT, costb, wk, wq, wv, wo, bpatx, wpat,
               evec, spat, zpat, out_d)
    nc.compile()
    return nc


def _build(nc, tc, qT, costb, wk, wq, wv, wo, bpatx, wpat, evec,
           spat, zpat, out_d):
    import contextlib
    ctx = contextlib.ExitStack()
    sb = ctx.enter_context
    # ---- static SBUF ----
    qT_sb = sb(nc.sbuf_tensor([128, 2 * 512], fmm))       # D-chunk c at cols 512c
    qTb = sb(nc.sbuf_tensor([128, 2 * NB * 128], fmm))    # [(D), (c,B,rep4,k32)]
    wk_sb = sb(nc.sbuf_tensor([128, 2 * 256], fmm))
    wq_sb = sb(nc.sbuf_tensor([128, 2 * 256], fmm))
    wv_sb = sb(nc.sbuf_tensor([128, 2 * 128], fmm))
    wo_sb = sb(nc.sbuf_tensor([128, 256], fmm))
    wpat_sb = sb(nc.sbuf_tensor([128, NSC * 128], fmm))
    evec_sb = sb(nc.sbuf_tensor([128, 36], f32))
    # Folded hidden-matmul operands. X: rhs rows [Q_h ; cost] per head pair,
    # cols (Bb, q). KBPA: lhsT rows [K-block bcast ; b/a pattern], cols
    # (Bb, sc, (s4,k32)).
    X01 = sb(nc.sbuf_tensor([128, NB * 512], fmm))
    X23 = sb(nc.sbuf_tensor([128, NB * 512], fmm))
    KBPA01 = sb(nc.sbuf_tensor([128, NB * NSC * 128], fmm))
    KBPA23 = sb(nc.sbuf_tensor([128, NB * NSC * 128], fmm))
    spat_sb = sb(nc.sbuf_tensor([128, 4], fmm))
    zpat_sb = sb(nc.sbuf_tensor([128, 128], f32))
    K_sb = sb(nc.sbuf_tensor([128, 512], fmm))            # [(4h,32d), k]
    Q_sb = sb(nc.sbuf_tensor([128, 512], fmm))            # [(4h,32d), q]
    Vr_sb = sb(nc.sbuf_tensor([128, NB * 128], fmm))      # [(4rep,32k), (h,d)] per B
    K_bc = sb(nc.sbuf_tensor([128, NB * NSC * 128], fmm))  # [(4h,32d), (B,sc,s4,k32)]
    hid_sb = sb(nc.sbuf_tensor([128, 3 * 4 * 512], fmm))  # 3 rounds x 4 tiles
    E_sb = sb(nc.sbuf_tensor([128, 3 * 512], fmm))        # 3 B-slots
    zr_sb = sb(nc.sbuf_tensor([128, 512], f32))           # rows 0-3 used
    zb_sb = sb(nc.sbuf_tensor([128, 512], f32))
    att_sb = sb(nc.sbuf_tensor([128, 512], fmm))
    out_sb = sb(nc.sbuf_tensor([128, 4 * 256], f32))
    # ---- PSUM (8 banks) ----
    hid_ps = [sb(nc.psum_tensor(f"hid_ps{i}", [128, 512], f32))
              for i in range(4)]
    sc_ps = [sb(nc.psum_tensor(f"sc_ps{i}", [128, 512], f32))
             for i in range(2)]
    att_ps = sb(nc.psum_tensor("att_ps", [128, 512], f32))
    sum_ps = sb(nc.psum_tensor("sum_ps", [128, 512], f32))
    zfill_sb = sb(nc.sbuf_tensor([32, 64], fmm))          # zero filler lhsT

    dma = nc.sync.dma_start
    nc.vector.memset(zfill_sb[:], 0.0)
    # preload the exp table set so the first mid-loop exp doesn't stall ACT
    nc.scalar.activation(zb_sb[0:1, 0:1], evec_sb[0:1, 0:1], AF.Exp)
    # ---- loads: critical-path inputs first, rest spread over the three
    # DMA rings (SP, GpSimd-SWDGE, Activation), chunked by Bb-quarter so
    # early rounds only wait for the first chunk ----
    for c in range(2):
        dma(qT_sb[:, 512 * c:512 * (c + 1)], qT[c])
        dma(wk_sb[:, 256 * c:256 * (c + 1)], wk[c])
        dma(wq_sb[:, 256 * c:256 * (c + 1)], wq[c])
    nc.scalar.dma_start(wv_sb[:, 0:128], wv[0])
    nc.scalar.dma_start(wv_sb[:, 128:256], wv[1])
    nc.scalar.dma_start(evec_sb[:], evec[:, :])
    nc.scalar.dma_start(spat_sb[:], spat[:, :])
    for s in range(NSC):
        nc.scalar.dma_start(wpat_sb[:, 128 * s:128 * (s + 1)], wpat[s])
    nc.gpsimd.dma_start(wo_sb[:], wo[:, :])
    nc.gpsimd.dma_start(zpat_sb[:], zpat[:, :])
    # cost (X01 rows 32-63 / 96-127 via HBM; X23 same-partition DVE copies)
    # and bpat (KBPA rows 32-63 / 96-127)
    for h in range(4):
        cs = slice(2048 * h, 2048 * (h + 1))
        for ri, rb in enumerate((32, 96)):
            q = (nc.sync, nc.gpsimd)[ri]
            q.dma_start(X01[rb:rb + 32, cs], costb[:, cs])
        for jj in range(4):
            Kb = KBPA01 if jj < 2 else KBPA23
            base = 64 * (jj % 2) + 32
            q = (nc.sync, nc.gpsimd)[(jj + 1) % 2]
            q.dma_start(Kb[base:base + 32, cs], bpatx[jj][:, cs])
    mm = nc.tensor.matmul

    # ---- qTb: s4-replicated qT for the V-build lhsT (lane-aligned) ----
    for c in range(2):
        nc.vector.tensor_copy(
            qTb[:, 2048 * c:2048 * (c + 1)]
                .rearrange("p (b s k) -> p b s k", s=4, k=32),
            qT_sb[:, 512 * c:512 * (c + 1)]
                .rearrange("p (b k) -> p b k", k=32)
                .unsqueeze(2).broadcast_to((128, NB, 4, 32)))

    # ---- K / Q projections: out [(4h,32d), n] (bf16 inputs, 1 cyc/row) ----
    for c in range(2):
        mm(hid_ps[0][:], wk_sb[:, 256 * c:256 * c + 128], qT_sb[:, 512 * c:512 * (c + 1)],
           start=(c == 0), stop=(c == 1), tile_position=(0, 0))
    nc.vector.tensor_copy(K_sb[:], hid_ps[0][:])
    for c in range(2):
        mm(hid_ps[1][:], wq_sb[:, 256 * c:256 * c + 128], qT_sb[:, 512 * c:512 * (c + 1)],
           start=(c == 0), stop=(c == 1), tile_position=(0, 0))
    nc.scalar.copy(Q_sb[:], hid_ps[1][:])

    # ---- K / Q bands of the folded-matmul operands. With the projection
    # column order permuted to (h0, h2, h1, h3), bands j0/j1 are
    # lane-aligned (same partitions) -> DVE copies; j2/j3 need a partition
    # shift -> DMA. gpos[jj] = source row group of head jj in K_sb/Q_sb.
    # All DVE work is chunked by Bb-quarter; quarters 1-3 are emitted
    # interleaved into the round stream so they fill DVE idle gaps instead
    # of blocking the queue on late ring chunks.
    gpos = (0, 2, 1, 3)

    def emit_dve_bands(h):
        cs = slice(2048 * h, 2048 * (h + 1))
        # (sc, s4)-broadcast K quarter
        nc.vector.tensor_copy(
            K_bc[:, cs].rearrange("p (b t s k) -> p b t s k", t=NSC, s=4, k=32),
            K_sb[:, 128 * h:128 * (h + 1)].rearrange("p (b k) -> p b k", k=32)
                .unsqueeze(2).unsqueeze(3).broadcast_to((128, 4, NSC, 4, 32)))
        for rb in (32, 96):
            nc.vector.tensor_copy(X23[rb:rb + 32, cs], X01[rb:rb + 32, cs])
        for jj in range(2):
            base = 64 * jj
            sr = 32 * gpos[jj]
            nc.vector.tensor_copy(
                X01[base:base + 32, cs].rearrange("p (b q) -> p b q", q=512),
                Q_sb[sr:sr + 32, :].unsqueeze(1).broadcast_to((32, 4, 512)))
            nc.vector.tensor_copy(KBPA01[base:base + 32, cs], K_bc[sr:sr + 32, cs])
        # j2/j3 K bands need a partition shift -> ring DMA, emitted after
        # this quarter's K_bc build (emission order is program order)
        for jj in (2, 3):
            base = 64 * (jj % 2)
            sr = 32 * gpos[jj]
            (nc.scalar if h == 0 else nc.gpsimd).dma_start(
                KBPA23[base:base + 32, cs], K_bc[sr:sr + 32, cs])

    for h in range(4):
        for jj in (2, 3):
            base = 64 * (jj % 2)
            sr = 32 * gpos[jj]
            dst = X23[base:base + 32, 2048 * h:2048 * (h + 1)] \
                .rearrange("p (b q) -> p b q", q=512)
            src = Q_sb[sr:sr + 32, :] \
                .unsqueeze(1).broadcast_to((32, 4, 512))
            (nc.scalar if h == 0 else nc.sync).dma_start(dst, src)
    emit_dve_bands(0)

    # ---- V replicated: Vr[B] [(4rep,32k), (h,d)] ----
    for g in range(4):           # 4 banks x 4 B each
        for i in range(4):
            Bb = 4 * g + i
            for c in range(2):
                lhsT = qTb[:, 2048 * c + 128 * Bb: 2048 * c + 128 * (Bb + 1)]
                mm(hid_ps[g][:, 128 * i:128 * (i + 1)], lhsT,
                   wv_sb[:, 128 * c:128 * (c + 1)],
                   start=(c == 0), stop=(c == 1), tile_position=(0, 0))
        if g % 2 == 0:
            nc.scalar.copy(Vr_sb[:, 512 * g:512 * (g + 1)], hid_ps[g][:])
        else:
            nc.vector.tensor_copy(Vr_sb[:, 512 * g:512 * (g + 1)], hid_ps[g][:])

    # ---- main loop ----
    def emit_round(Bb, sc):
        slot = (Bb * NSC + sc) % 3
        hbase = 2048 * slot
        # Filler matmuls: occupy the PE array during the stall window
        # (waiting on the previous round's PSUM evacuation) so the HAM
        # clock gate keeps the PE at full clock. Zero weights into unused
        # rows 32-127 of the sum_ps bank: no data deps, nothing accumulates,
        # rows 0-3 (the real running sum) are untouched (has_written is
        # per-element and start=False never clears the bank).
        for f in range(MMFILL):
            mm(sum_ps[64:128, :], zfill_sb[:], Q_sb[0:32, :],
               start=False, stop=False, tile_position=(0, 64),
               skip_group_check=True)
        for f in range(NFILL):
            nc.tensor.ldweights(K_bc[0:32, 128 * Bb:128 * (Bb + 1)],
                                tile_position=(0, 0))
        cb = 128 * (NSC * Bb + sc)
        for j in range(4):
            Xb = X01 if j < 2 else X23
            Kb = KBPA01 if j < 2 else KBPA23
            base = 64 * (j % 2)
            mm(hid_ps[j][:], Kb[base:base + 64, cb:cb + 128],
               Xb[base:base + 64, 512 * Bb:512 * (Bb + 1)],
               start=True, stop=True, tile_position=(base, 0))
        for j in range(4):
            dst = hid_sb[:, hbase + 512 * j: hbase + 512 * (j + 1)]
            if _act_form(j, sc):
                t = 2 * (sc * 4 + j)
                nc.scalar.activation(dst, hid_ps[j][:], AF.Relu,
                                     bias=evec_sb[:, t + 1:t + 2],
                                     scale=evec_sb[:, t:t + 1])
            else:
                t = 32 if j == 1 else 2 * (sc * 4 + j)
                nc.vector.tensor_scalar(dst, hid_ps[j][:],
                                        evec_sb[:, t:t + 1],
                                        evec_sb[:, t + 1:t + 2],
                                        op0=ALU.mult, op1=ALU.max)

    def emit_mix2(Bb, sc):
        slot = (Bb * NSC + sc) % 3
        hbase = 2048 * slot
        sps = sc_ps[Bb % 2]
        for j in range(4):
            lhsT = wpat_sb[:, 128 * sc + 32 * j: 128 * sc + 32 * (j + 1)]
            mm(sps[32 * j:32 * j + 32, :], lhsT,
               hid_sb[:, hbase + 512 * j: hbase + 512 * (j + 1)],
               start=(sc == 0), stop=(sc == NSC - 1), tile_position=(0, 32 * j),
               skip_group_check=True)

    def emit_exp(Bb):
        nc.scalar.activation(E_sb[:, 512 * (Bb % 3):512 * (Bb % 3 + 1)],
                             sc_ps[Bb % 2][:], AF.Exp)

    def emit_av(Bb):
        for j in range(4):
            mm(att_ps[32 * j:32 * j + 32, :],
               Vr_sb[32 * j:32 * j + 32, 128 * Bb + 32 * j:128 * Bb + 32 * (j + 1)],
               E_sb[32 * j:32 * j + 32, 512 * (Bb % 3):512 * (Bb % 3 + 1)],
               start=(Bb == 0), stop=(Bb == NB - 1), tile_position=(32 * j, 32 * j),
               skip_group_check=True)
        mm(sum_ps[0:4, :], spat_sb[:],
           E_sb[:, 512 * (Bb % 3):512 * (Bb % 3 + 1)],
           start=(Bb == 0), stop=(Bb == NB - 1), tile_position=(0, 0),
           skip_group_check=True)

    # software pipeline: mix2 lags rounds by one step; exp after mix2(sc=3);
    # AV lags exp by one B.
    steps = [(Bb, sc) for Bb in range(NB) for sc in range(NSC)]
    dve_band_at = {4: 1, 10: 2, 24: 3}
    for idx, (Bb, sc) in enumerate(steps):
        if idx in dve_band_at:
            emit_dve_bands(dve_band_at[idx])
        emit_round(Bb, sc)
        if idx >= 1:
            pB, psc = steps[idx - 1]
            emit_mix2(pB, psc)
            if psc == NSC - 1:
                emit_exp(pB)
                if pB >= 1:
                    emit_av(pB - 1)
    emit_mix2(*steps[-1])
    emit_exp(NB - 1)
    emit_av(NB - 2)
    emit_av(NB - 1)

    # ---- tail: normalize + output projection ----
    nc.vector.reciprocal_approx_fast(zr_sb[0:4, :], sum_ps[0:4, :])
    mm(sc_ps[0][:], zpat_sb[0:4, 0:128], zr_sb[0:4, :],
       start=True, stop=True, tile_position=(0, 0))
    nc.scalar.copy(zb_sb[:], sc_ps[0][:])
    nc.vector.tensor_tensor(att_sb[:], att_ps[:], zb_sb[:], op=ALU.mult)
    for qc in range(4):
        ps = sc_ps[qc % 2]
        half = 256 * (qc // 2)
        mm(ps[:, half:half + 256], att_sb[:, 128 * qc:128 * (qc + 1)],
           wo_sb[:], start=True, stop=True, tile_position=(0, 0))
        if qc % 2 == 0:
            nc.vector.tensor_copy(out_sb[:, 256 * qc:256 * (qc + 1)], ps[:, half:half + 256])
        else:
            nc.scalar.copy(out_sb[:, 256 * qc:256 * (qc + 1)], ps[:, half:half + 256])
        oq = (nc.sync, nc.gpsimd, nc.sync, nc.scalar)[qc]
        oq.dma_start(out_d[128 * qc:128 * (qc + 1), :], out_sb[:, 256 * qc:256 * (qc + 1)])
    ctx.close()


# --------------------------------------------------------------------------
# host-side input prep
# --------------------------------------------------------------------------
def make_core_inputs(inputs, core):
    b, quad = core // 2, core % 2
    mmdt = ml_dtypes.bfloat16 if MM_FAST else np.float32
    queries = inputs["queries"][b]            # [512, 256]
    cost = inputs["cost_mat"][b]              # [512, 512]
    a = inputs["mix1_w"][:, 0, :]             # [H, MS]
    bb = inputs["mix1_w"][:, 1, :]
    cc = inputs["mix1_b"]                     # [H, MS]
    w2 = inputs["mix2_w"][:, :, 0]            # [H, MS]
    hs = slice(quad * 4 * DK, (quad + 1) * 4 * DK)

    qT = np.ascontiguousarray(queries.T).reshape(2, 128, 512)
    costT = np.ascontiguousarray(cost.T)      # [k, q]
    costb = np.ascontiguousarray(
        costT.reshape(NB, 32, 512).transpose(1, 0, 2)).reshape(32, NB * 512)
    wk = np.ascontiguousarray(inputs["Wk"]).reshape(2, 128, 256)
    wq = (np.ascontiguousarray(inputs["Wq"]) * (DK ** -0.5)).astype(np.float32).reshape(2, 128, 256)
    # K/Q proj in the program use cols [256c : 256c+128] -> must be the quad's
    # 128 cols: bake quad slice so lhsT slice [*, :128] is the quad cols.
    wk = np.ascontiguousarray(wk[:, :, hs])   # [2,128,128]
    wq = np.ascontiguousarray(wq[:, :, hs])
    # permute proj outputs to (h0, h2, h1, h3) so the j0/j1 band copies are
    # lane-aligned (see gpos in the device program)
    perm = np.r_[0:32, 64:96, 32:64, 96:128]
    wk = wk[:, :, perm]
    wq = wq[:, :, perm]
    wk = np.concatenate([wk, np.zeros_like(wk)], axis=2)  # pad back to 256 cols
    wq = np.concatenate([wq, np.zeros_like(wq)], axis=2)
    wv = np.ascontiguousarray(inputs["Wv"][:, hs]).reshape(2, 128, 128)
    wo = np.ascontiguousarray(inputs["Wo"][hs, :])        # [128, 256]

    bpx = np.zeros((4, 32, NSC, 128), np.float32)  # affine rows per (j, sc)
    wpat = np.zeros((NSC, 128, 128), np.float32)
    evec = np.zeros((128, 36), np.float32)
    for sc in range(NSC):
        for j in range(4):
            h = quad * 4 + j
            for si in range(4):
                s = sc * 4 + si
                ah, bh, ch, wh = a[h, s], bb[h, s], cc[h, s], w2[h, s]
                rows = np.arange(32)
                # affine lhsT rows (cost k') x cols (si,kk): b/a on the diag
                bpx[j, rows, sc, 32 * si + rows] = bh / ah
                p = 32 * si + rows                      # hidden partition idx
                if _act_form(j, sc):
                    evec[p, 2 * (sc * 4 + j)] = ah
                    evec[p, 2 * (sc * 4 + j) + 1] = ch
                    wpat[sc, p, 32 * j + rows] = wh
                else:
                    evec[p, 2 * (sc * 4 + j)] = np.sign(ah)
                    evec[p, 2 * (sc * 4 + j) + 1] = -ch / abs(ah)
                    wpat[sc, p, 32 * j + rows] = wh * abs(ah)
    # replicate the (sc, col) pattern over the NB column blocks
    bpatx = np.broadcast_to(bpx[:, :, None], (4, 32, NB, NSC, 128))
    bpatx = np.ascontiguousarray(bpatx).reshape(4, 32, NB * NSC * 128)
    spat = np.zeros((128, 4), np.float32)
    for j in range(4):
        spat[32 * j:32 * (j + 1), j] = 1.0
    zpat = np.zeros((128, 128), np.float32)
    for j in range(4):
        zpat[j, 32 * j:32 * (j + 1)] = 1.0
    return dict(qT=qT.astype(mmdt),
                costb=costb.astype(mmdt),
                wk=wk.astype(mmdt), wq=wq.astype(mmdt), wv=wv.astype(mmdt),
                wo=np.ascontiguousarray(wo).astype(mmdt),
                bpatx=bpatx.astype(mmdt), wpat=wpat.astype(mmdt),
                evec=evec,
                spat=spat.astype(mmdt), zpat=zpat)


def kernel(**inputs):
    global _last_results
    inputs = {k: np.asarray(v, np.float32) for k, v in inputs.items()}
    if "nc" not in _compiled:
        _compiled["nc"] = build_program()
    nc = _compiled["nc"]
    in_maps = [make_core_inputs(inputs, core) for core in range(8)]
    trace = bool(os.environ.get("MSK_TRACE"))
    if trace:
        _install_ntff_hook()
    res = run_bass_kernel_spmd(nc, in_maps, list(range(8)), trace=trace)
    _last_results = res
    out = np.zeros((B_, L, D), np.float32)
    for core in range(8):
        out[core // 2] += res.results[core]["out"]
    return out
